# revision 1
# baseline (speedup 1.0000x reference)
"""Linear-attention Trainium2 kernel (8 NeuronCores, SPMD).

Sharding: batch (4) x head-group (2). Core i handles batch i//2, heads
[8*(i%2), 8*(i%2)+8). Each core computes its partial output through Wo;
the host sums the two partials per batch and adds bo.

Per-core dataflow (all matmuls in float32r):
  xT = x[b].T                                   [1024, 4096]   (host transpose)
  Q^T = Wq_g^T-contract xT  (PE, f on parts)    [512, 4096]    d on partitions
  expQ^T = exp(Q^T + bq)    (ACT, bias/part)
  sQ    = blockdiag-ones matmul colsums         [8, 4096]
  K     = xT^T-contract Wk_g (natural layout)   [4096, 512]    s on partitions
  expK  = exp(K + bk)       (ACT; bias via k=1 outer-product matmul)
  V'    = (V + bv) * 1/rowsum(expK) per head    (DVE tensor_scalar per head)
  KV_h  = expK_h^T @ V'_h   (PE, accumulated in PSUM over all of S)
  out^T_h = KV_h^T-contract expQ^T_h, then * (1/sQ) via DMA-broadcast + DVE
  y_partial = out^T^T-contract Wo_g             [4096, 1024]
"""

import numpy as np

B, S, DM, H = 4, 4096, 1024, 16
HD = 64
GROUPS = 2
DLOC = DM // GROUPS   # 512 channels per core
HLOC = H // GROUPS    # 8 heads per core
NCORES = B * GROUPS   # 8
SC = 512              # sequence chunk


def make_consts():
    ones1 = np.ones((1, 128), np.float32)
    ones8 = np.zeros((128, 4 * HLOC), np.float32)
    for dt_ in range(4):  # pair-tile index
        for sub in range(2):
            ones8[64 * sub:64 * (sub + 1), dt_ * HLOC + 2 * dt_ + sub] = 1.0
    return ones1, ones8


def kv_region(h):
    """(half, row_base, col_base) of KV_h inside kv psum tile [128, 2, 512]."""
    return h // 4, 64 * (h % 2), 256 * ((h // 2) % 2) + 64 * (h % 4)


def build_bass(S_=S, n_devices=NCORES, repeat=1, dbg=False):
    from contextlib import ExitStack
    import concourse.bass as bass
    import concourse.bacc as bacc
    import concourse.mybir as mybir
    import concourse.tile as tile

    f32 = mybir.dt.float32
    f32r = mybir.dt.float32r
    Exp = mybir.ActivationFunctionType.Exp
    X = mybir.AxisListType.X

    NCH = S_ // SC        # sequence chunks
    NPAIR = DLOC // 128   # 4 pair-tiles (2 heads each)
    NST = S_ // 128       # sequence tiles

    nc = bacc.Bacc("TRN2", target_bir_lowering=False, debug=False,
                   num_devices=n_devices)
    xT = nc.dram_tensor("xT", [DM, S_], f32r, kind="ExternalInput").ap()
    wq = nc.dram_tensor("wq", [DM, DLOC], f32r, kind="ExternalInput").ap()
    wk = nc.dram_tensor("wk", [DM, DLOC], f32r, kind="ExternalInput").ap()
    wv = nc.dram_tensor("wv", [DM, DLOC], f32r, kind="ExternalInput").ap()
    wo = nc.dram_tensor("wo", [DLOC, DM], f32r, kind="ExternalInput").ap()
    bq = nc.dram_tensor("bq", [DLOC], f32, kind="ExternalInput").ap()
    bk = nc.dram_tensor("bk", [1, DLOC], f32r, kind="ExternalInput").ap()
    bv = nc.dram_tensor("bv", [1, DLOC], f32r, kind="ExternalInput").ap()
    ones1 = nc.dram_tensor("ones1", [1, 128], f32r, kind="ExternalInput").ap()
    ones8 = nc.dram_tensor("ones8", [128, 4 * HLOC], f32r,
                           kind="ExternalInput").ap()
    y = nc.dram_tensor("y", [S_, DM], f32, kind="ExternalOutput").ap()
    NPAIR_ = DLOC // 128
    if dbg:
        d_expqt = nc.dram_tensor("d_expqt", [128, NPAIR_, S_], f32,
                                 kind="ExternalOutput").ap()
        d_recq = nc.dram_tensor("d_recq", [HLOC, S_], f32,
                                kind="ExternalOutput").ap()
        d_kv = nc.dram_tensor("d_kv", [128, 2, 512], f32,
                              kind="ExternalOutput").ap()
        d_ot = nc.dram_tensor("d_ot", [S_ // SC, 128, NPAIR_, SC], f32,
                              kind="ExternalOutput").ap()

    xTr = xT.rearrange("(tf p) s -> p tf s", p=128)

    def body(tc):
        ctx = ExitStack()
        with ctx:
            cons = ctx.enter_context(tc.tile_pool(name="cons", bufs=1))
            persist = ctx.enter_context(tc.tile_pool(name="persist", bufs=1))
            kvpsp = ctx.enter_context(
                tc.tile_pool(name="kvps", bufs=1, space="PSUM"))

            bqT = cons.tile([128, NPAIR], f32)
            nc.sync.dma_start(out=bqT, in_=bq.rearrange("(t p) -> p t", p=128))
            bk_sb = cons.tile([1, DLOC], f32r)
            nc.sync.dma_start(out=bk_sb, in_=bk)
            bv_sb = cons.tile([1, DLOC], f32r)
            nc.sync.dma_start(out=bv_sb, in_=bv)
            o1 = cons.tile([1, 128], f32r)
            nc.sync.dma_start(out=o1, in_=ones1)
            o8 = cons.tile([128, 4 * HLOC], f32r)
            nc.sync.dma_start(out=o8, in_=ones8)

            expQT = persist.tile([128, NPAIR, S_], f32r)
            recq = persist.tile([HLOC, S_], f32r)
            kvsb = persist.tile([128, 2, 512], f32r)
            kvA = kvpsp.tile([128, 512], f32, tag="kvA")
            kvB = kvpsp.tile([128, 512], f32, tag="kvB")

            # ---------------- phase 1 ----------------
            with ExitStack() as p1:
                wpool = p1.enter_context(tc.tile_pool(name="wqkv", bufs=1))
                xpool = p1.enter_context(tc.tile_pool(name="xc", bufs=2))
                ekpool = p1.enter_context(tc.tile_pool(name="ek", bufs=4))
                vnpool = p1.enter_context(tc.tile_pool(name="vn", bufs=4))
                smpool = p1.enter_context(tc.tile_pool(name="sm", bufs=4))
                qpsp = p1.enter_context(
                    tc.tile_pool(name="qps", bufs=2, space="PSUM"))
                sqpsp = p1.enter_context(
                    tc.tile_pool(name="sqps", bufs=1, space="PSUM"))
                pkvp = p1.enter_context(
                    tc.tile_pool(name="pkv", bufs=3, space="PSUM"))

                wq_sb = wpool.tile([128, 8, DLOC], f32r, tag="wq")
                nc.sync.dma_start(
                    out=wq_sb, in_=wq.rearrange("(tf p) d -> p tf d", p=128))
                wk_sb = wpool.tile([128, 8, DLOC], f32r, tag="wk")
                nc.sync.dma_start(
                    out=wk_sb, in_=wk.rearrange("(tf p) d -> p tf d", p=128))
                wv_sb = wpool.tile([128, 8, DLOC], f32r, tag="wv")
                nc.sync.dma_start(
                    out=wv_sb, in_=wv.rearrange("(tf p) d -> p tf d", p=128))

                for c in range(NCH):
                    xc = xpool.tile([128, 8, SC], f32r, tag="xc")
                    nc.sync.dma_start(out=xc,
                                      in_=xTr[:, :, c * SC:(c + 1) * SC])
                    # Q^T pair-tiles + exp + column sums
                    sqps = sqpsp.tile([HLOC, SC], f32, tag="sq")
                    for dt_ in range(NPAIR):
                        qps = qpsp.tile([128, SC], f32, tag="q")
                        for tf in range(8):
                            nc.tensor.matmul(
                                qps, wq_sb[:, tf, dt_ * 128:(dt_ + 1) * 128],
                                xc[:, tf, :],
                                start=(tf == 0), stop=(tf == 7))
                        eq = expQT[:, dt_, c * SC:(c + 1) * SC]
                        nc.scalar.activation(eq, qps, Exp,
                                             bias=bqT[:, dt_:dt_ + 1],
                                             scale=1.0)
                        nc.tensor.matmul(
                            sqps, o8[:, dt_ * HLOC:(dt_ + 1) * HLOC], eq,
                            start=(dt_ == 0), stop=(dt_ == NPAIR - 1))
                    with nc.allow_low_precision(reason="f32r rounding ok"):
                        nc.vector.reciprocal(
                            recq[:, c * SC:(c + 1) * SC], sqps)

                    # K / V / KV per 128-row sequence tile
                    for t in range(4):
                        st = c * 4 + t
                        kps = pkvp.tile([128, DLOC], f32, tag="pkv")
                        for tf in range(8):
                            nc.tensor.matmul(
                                kps, xc[:, tf, t * 128:(t + 1) * 128],
                                wk_sb[:, tf, :],
                                start=(tf == 0), stop=False)
                        nc.tensor.matmul(kps, o1, bk_sb,
                                         start=False, stop=True)
                        ek = ekpool.tile([128, DLOC], f32r, tag="ek")
                        nc.scalar.activation(ek, kps, Exp)
                        sk = smpool.tile([128, HLOC], f32, tag="sk")
                        nc.vector.reduce_sum(
                            sk, ek.rearrange("p (h e) -> p h e", e=HD), axis=X)
                        rk = smpool.tile([128, HLOC], f32, tag="rk")
                        nc.vector.reciprocal(rk, sk)

                        vps = pkvp.tile([128, DLOC], f32, tag="pkv")
                        for tf in range(8):
                            nc.tensor.matmul(
                                vps, xc[:, tf, t * 128:(t + 1) * 128],
                                wv_sb[:, tf, :],
                                start=(tf == 0), stop=False)
                        nc.tensor.matmul(vps, o1, bv_sb,
                                         start=False, stop=True)
                        vn = vnpool.tile([128, DLOC], f32r, tag="vn")
                        rkb = bass.AP(
                            tensor=rk.tensor, offset=rk.offset,
                            ap=[list(rk.ap[0]), [1, HLOC], [0, HD]])
                        nc.vector.tensor_tensor(
                            out=vn.rearrange("p (h e) -> p h e", e=HD),
                            in0=vps.rearrange("p (h e) -> p h e", e=HD),
                            in1=rkb, op=mybir.AluOpType.mult)

                        first, last = (st == 0), (st == NST - 1)
                        for dst, lo, hi in ((kvA, 0, 256), (kvB, 256, 512)):
                            # start=True clears the whole 2KB psum row of
                            # every partition it writes, so only the first
                            # matmul into each bank may carry it.
                            nc.tensor.matmul(dst[:, 0:256],
                                             ek[:, lo:lo + 128],
                                             vn[:, lo:hi],
                                             start=first, stop=False,
                                             skip_group_check=True)
                            nc.tensor.matmul(dst[:, 256:512],
                                             ek[:, lo + 128:lo + 256],
                                             vn[:, lo:hi],
                                             start=False, stop=last,
                                             skip_group_check=True)

            # ---------------- phase 2 ----------------
            with ExitStack() as p2:
                wopool = p2.enter_context(tc.tile_pool(name="wo", bufs=1))
                otpool = p2.enter_context(tc.tile_pool(name="ot", bufs=2))
                rqpool = p2.enter_context(tc.tile_pool(name="rq", bufs=8))
                ysbpool = p2.enter_context(tc.tile_pool(name="ysb", bufs=3))
                opsp = p2.enter_context(
                    tc.tile_pool(name="ops", bufs=2, space="PSUM"))
                ypsp = p2.enter_context(
                    tc.tile_pool(name="yps", bufs=4, space="PSUM"))

                wo_sb = wopool.tile([128, NPAIR, DM], f32r)
                nc.sync.dma_start(
                    out=wo_sb, in_=wo.rearrange("(t p) j -> p t j", p=128))
                # zero the cross-head blocks so each 128x128 pair block of
                # KV becomes exactly block-diagonal, usable whole as lhsT
                for kvp in (kvA, kvB):
                    nc.vector.memset(kvp[0:64, 64:128], 0.0)
                    nc.vector.memset(kvp[64:128, 0:64], 0.0)
                    nc.vector.memset(kvp[0:64, 448:512], 0.0)
                    nc.vector.memset(kvp[64:128, 384:448], 0.0)
                nc.scalar.copy(kvsb[:, 0, :], kvA)
                nc.scalar.copy(kvsb[:, 1, :], kvB)
                if dbg:
                    nc.sync.dma_start(out=d_expqt, in_=expQT.bitcast(f32))
                    nc.sync.dma_start(out=d_recq, in_=recq.bitcast(f32))
                    nc.sync.dma_start(out=d_kv, in_=kvsb.bitcast(f32))

                for c in range(NCH):
                    otc = otpool.tile([128, NPAIR, SC], f32r, tag="otc")
                    for p_ in range(NPAIR):
                        ops = opsp.tile([128, SC], f32, tag="ops")
                        blk = kvsb[:, p_ // 2, 384 * (p_ % 2):
                                   384 * (p_ % 2) + 128]
                        nc.tensor.matmul(ops, blk,
                                         expQT[:, p_, c * SC:(c + 1) * SC],
                                         start=True, stop=True)
                        rqb = rqpool.tile([128, SC], f32r, tag="rqb")
                        for sub in range(2):
                            h = 2 * p_ + sub
                            src_ = recq[h:h + 1, c * SC:(c + 1) * SC]
                            bc = bass.AP(
                                tensor=src_.tensor, offset=src_.offset,
                                ap=[list(src_.ap[0]), [0, 64]]
                                + [list(d) for d in src_.ap[1:]])
                            nc.sync.dma_start(
                                out=rqb[64 * sub:64 * (sub + 1), :], in_=bc)
                        nc.vector.tensor_mul(otc[:, p_, :], ops, rqb)
                    if dbg:
                        nc.sync.dma_start(out=d_ot[c], in_=otc.bitcast(f32))
                    for t in range(4):
                        ysb = ysbpool.tile([128, 2, 512], f32, tag="ysb")
                        for jh in range(2):
                            yps = ypsp.tile([128, 512], f32, tag="yps")
                            for ct in range(NPAIR):
                                nc.tensor.matmul(
                                    yps,
                                    otc[:, ct, t * 128:(t + 1) * 128],
                                    wo_sb[:, ct, jh * 512:(jh + 1) * 512],
                                    start=(ct == 0), stop=(ct == NPAIR - 1))
                            nc.scalar.copy(ysb[:, jh, :], yps)
                        row = (c * 4 + t) * 128
                        nc.sync.dma_start(
                            out=y[row:row + 128, :].rearrange(
                                "p (a b) -> p a b", a=2),
                            in_=ysb)

    with tile.TileContext(nc) as tc:
        if repeat == 1:
            body(tc)
        else:
            for _ in range(repeat):
                body(tc)
    nc.compile()
    return nc


def shard_inputs(x, Wq, bq, Wk, bk, Wv, bv, Wo, S_=S):
    ones1, ones8 = make_consts()
    f = np.float32
    in_maps = []
    for core in range(NCORES):
        b, g = core // GROUPS, core % GROUPS
        sl = slice(g * DLOC, (g + 1) * DLOC)
        in_maps.append({
            "xT": np.ascontiguousarray(np.asarray(x)[b, :S_, :].T, dtype=f),
            "wq": np.ascontiguousarray(np.asarray(Wq)[:, sl], dtype=f),
            "wk": np.ascontiguousarray(np.asarray(Wk)[:, sl], dtype=f),
            "wv": np.ascontiguousarray(np.asarray(Wv)[:, sl], dtype=f),
            "wo": np.ascontiguousarray(np.asarray(Wo)[sl, :], dtype=f),
            "bq": np.asarray(bq)[sl].astype(f),
            "bk": np.asarray(bk)[sl].astype(f)[None, :],
            "bv": np.asarray(bv)[sl].astype(f)[None, :],
            "ones1": ones1,
            "ones8": ones8,
        })
    return in_maps


_NC_CACHE = {}


def _get_nc():
    if "nc" not in _NC_CACHE:
        _NC_CACHE["nc"] = build_bass()
    return _NC_CACHE["nc"]


def kernel(x, Wq, bq, Wk, bk, Wv, bv, Wo, bo):
    from concourse.bass_utils import run_bass_kernel_spmd
    nc = _get_nc()
    in_maps = shard_inputs(x, Wq, bq, Wk, bk, Wv, bv, Wo)
    res = run_bass_kernel_spmd(nc, in_maps, list(range(NCORES)))
    parts = [res.results[i]["y"] for i in range(NCORES)]
    out = np.stack([parts[2 * b] + parts[2 * b + 1] for b in range(B)])
    out += np.asarray(bo, dtype=np.float32)
    return out.astype(np.float32)


def oracle_core(inp, S_=S):
    """Numpy mirror of the per-core computation, for debugging."""
    xT = inp["xT"].astype(np.float64)
    Q = xT.T @ inp["wq"] + inp["bq"]
    K = xT.T @ inp["wk"] + inp["bk"][0]
    V = xT.T @ inp["wv"] + inp["bv"][0]
    out = np.zeros((S_, DLOC))
    for h in range(HLOC):
        sl = slice(h * HD, (h + 1) * HD)
        eq, ek = np.exp(Q[:, sl]), np.exp(K[:, sl])
        qh = eq / eq.sum(-1, keepdims=True)
        kh = ek / ek.sum(-1, keepdims=True)
        out[:, sl] = qh @ (kh.T @ V[:, sl])
    return (out @ inp["wo"]).astype(np.float32)



# revision 12
# speedup vs baseline: 1.3143x; 1.3143x over previous
"""Linear-attention Trainium2 kernel (8 NeuronCores, SPMD), v2.

Sharding: batch (4) x head-group (2). Core i handles batch i//2, heads
[8*(i%2), 8*(i%2)+8). Each core computes its partial y through Wo; the
host sums the two partials per batch and adds bo.

Math restructure vs v1: y = diag(recq) . expQ @ M with M = KV @ Wo_g,
so the per-head out-matmuls and all recq broadcast DMAs disappear; the
Q-softmax normalization is a block-diag-ones matmul (replicated per-head
column sums) + DVE reciprocal + DVE multiply on expQT.

All matmul operands are bf16 (same PE rate as f32r in the cost model,
half the DMA bytes); PSUM accumulation stays f32. End-to-end rel-err vs
the f32 reference is ~4e-3 (gate 2e-2).

Per-core dataflow:
  pass A (per 512-chunk c):  x chunk -> resident SBUF
    K = x_c @ Wk_g          (PE, 8 contract steps per 128-seq tile)
    ek = exp(K)             (ACT -> bf16)
    rk = 1/rowsum_h(ek)     (DVE reduce + reciprocal)
    V = x_c @ Wv_g          (PE)
    vn = V * rk             (DVE, per-head broadcast)
    KVt += vn^T-contract ek (PE, 4 pair-blocks into ONE psum bank)
  pass B: zero off-diag 64x64 blocks, KVt -> SBUF bf16,
    M_p = KVt_p-block^T-contract Wo rows  (8 mms), M -> SBUF bf16
  pass C (per chunk c):
    Q^T = Wq_g^T-contract x_c (PE), eq = exp(Q^T + bq) (ACT -> bf16)
    sq = Lones^T-contract eq  (PE, replicated per-head sums)
    eqn = eq * (1/sq)         (DVE recip + mult -> bf16)
    y_tile = eqn^T-contract M (PE psum) -> SBUF f32 -> DRAM
"""

import numpy as np

B, S, DM, H = 4, 4096, 1024, 16
HD = 64
GROUPS = 2
DLOC = DM // GROUPS   # 512 channels per core
HLOC = H // GROUPS    # 8 heads per core
NCORES = B * GROUPS   # 8
SC = 512              # sequence chunk
NPAIR = DLOC // 128   # 4 head-pair tiles


def make_consts():
    # block-diag ones [128,128]: L[d,p]=1 iff d,p in same 64-block
    L = np.zeros((128, 128), np.float32)
    L[:64, :64] = 1.0
    L[64:, 64:] = 1.0
    ones1 = np.ones((1, 128), np.float32)
    return L, ones1


def build_bass(S_=S, n_devices=NCORES, repeat=1, dbg=False, kv_bias=False):
    from contextlib import ExitStack
    import concourse.bass as bass
    import concourse.bacc as bacc
    import concourse.mybir as mybir
    import concourse.tile as tile

    f32 = mybir.dt.float32
    bf16 = mybir.dt.bfloat16
    Exp = mybir.ActivationFunctionType.Exp
    X = mybir.AxisListType.X

    NCH = S_ // SC        # sequence chunks
    NST = S_ // 128       # 128-row sequence tiles

    nc = bacc.Bacc("TRN2", target_bir_lowering=False, debug=False,
                   num_devices=n_devices)
    xT = nc.dram_tensor("xT", [DM, S_], bf16, kind="ExternalInput").ap()
    wq = nc.dram_tensor("wq", [DM, DLOC], bf16, kind="ExternalInput").ap()
    wk = nc.dram_tensor("wk", [DM, DLOC], bf16, kind="ExternalInput").ap()
    wv = nc.dram_tensor("wv", [DM, DLOC], bf16, kind="ExternalInput").ap()
    wo = nc.dram_tensor("wo", [DLOC, DM], bf16, kind="ExternalInput").ap()
    bq = nc.dram_tensor("bq", [DLOC], f32, kind="ExternalInput").ap()
    lones = nc.dram_tensor("lones", [128, 128], bf16,
                           kind="ExternalInput").ap()
    if kv_bias:
        ones1 = nc.dram_tensor("ones1", [1, 128], bf16,
                               kind="ExternalInput").ap()
        bk = nc.dram_tensor("bk", [1, DLOC], bf16, kind="ExternalInput").ap()
        bv = nc.dram_tensor("bv", [1, DLOC], bf16, kind="ExternalInput").ap()
    y = nc.dram_tensor("y", [S_, DM], f32, kind="ExternalOutput").ap()
    if dbg:
        d_eqn = nc.dram_tensor("d_eqn", [128, NPAIR, S_], bf16,
                               kind="ExternalOutput").ap()
        d_kvt = nc.dram_tensor("d_kvt", [128, 512], f32,
                               kind="ExternalOutput").ap()
        d_m = nc.dram_tensor("d_m", [128, NPAIR, DM], bf16,
                             kind="ExternalOutput").ap()

    xTr = xT.rearrange("(tf p) s -> p tf s", p=128)

    def body(tc):
        ctx = ExitStack()
        with ctx:
            cons = ctx.enter_context(tc.tile_pool(name="cons", bufs=1))
            persist = ctx.enter_context(tc.tile_pool(name="persist", bufs=1))

            # constants / weights
            bqT = cons.tile([128, NPAIR], f32)
            nc.sync.dma_start(out=bqT, in_=bq.rearrange("(t p) -> p t", p=128))
            lo_sb = cons.tile([128, 128], bf16)
            nc.sync.dma_start(out=lo_sb, in_=lones)
            if kv_bias:
                o1 = cons.tile([1, 128], bf16)
                nc.sync.dma_start(out=o1, in_=ones1)
                bk_sb = cons.tile([1, DLOC], bf16)
                nc.sync.dma_start(out=bk_sb, in_=bk)
                bv_sb = cons.tile([1, DLOC], bf16)
                nc.sync.dma_start(out=bv_sb, in_=bv)

            wk_sb = persist.tile([128, 8, DLOC], bf16, tag="wk")
            nc.sync.dma_start(
                out=wk_sb, in_=wk.rearrange("(tf p) d -> p tf d", p=128))
            wv_sb = persist.tile([128, 8, DLOC], bf16, tag="wv")
            nc.sync.dma_start(
                out=wv_sb, in_=wv.rearrange("(tf p) d -> p tf d", p=128))
            wq_sb = persist.tile([128, 8, DLOC], bf16, tag="wq")
            nc.sync.dma_start(
                out=wq_sb, in_=wq.rearrange("(tf p) d -> p tf d", p=128))
            wo_sb = persist.tile([128, NPAIR, DM], bf16, tag="wo")
            nc.sync.dma_start(
                out=wo_sb, in_=wo.rearrange("(t p) j -> p t j", p=128))

            xsb = persist.tile([128, 8, S_], bf16, tag="xsb")
            m_sb = persist.tile([128, NPAIR, DM], bf16, tag="msb")
            kvt_sb = persist.tile([128, 512], bf16, tag="kvtsb")

            kvctx = ExitStack()
            kvpsp = kvctx.enter_context(
                tc.tile_pool(name="kvps", bufs=1, space="PSUM"))
            kvt = kvpsp.tile([128, 512], f32, tag="kvt")

            # ---------------- pass A: K/V -> KVt ----------------
            with ExitStack() as pA:
                ekpool = pA.enter_context(tc.tile_pool(name="ek", bufs=4))
                vnpool = pA.enter_context(tc.tile_pool(name="vn", bufs=4))
                smpool = pA.enter_context(tc.tile_pool(name="sm", bufs=4))
                pkvp = pA.enter_context(
                    tc.tile_pool(name="pkv", bufs=4, space="PSUM"))

                # KVt[e, d] += vn^T @ ek, 4 head-pair blocks per seq tile.
                # start=True on the first mm clears the whole 2KB psum row
                # of all 128 partitions; later pairs must not carry it.
                # Emitted one seq-tile late so the ek/vn ACT+DVE chain of
                # tile st resolves while tile st+1's K matmuls run.
                def emit_kvt(st, ek, vn):
                    for p_ in range(NPAIR):
                        nc.tensor.matmul(
                            kvt[:, p_ * 128:(p_ + 1) * 128],
                            vn[:, p_ * 128:(p_ + 1) * 128],
                            ek[:, p_ * 128:(p_ + 1) * 128],
                            start=(st == 0 and p_ == 0),
                            stop=(st == NST - 1),
                            skip_group_check=True)

                pend = None
                for st in range(NST):
                    if st % 4 == 0:
                        c = st // 4
                        nc.sync.dma_start(
                            out=xsb[:, :, c * SC:(c + 1) * SC],
                            in_=xTr[:, :, c * SC:(c + 1) * SC])
                    xs = xsb[:, :, st * 128:(st + 1) * 128]
                    kps = pkvp.tile([128, DLOC], f32, tag="pkv")
                    for tf in range(8):
                        nc.tensor.matmul(
                            kps, xs[:, tf, :], wk_sb[:, tf, :],
                            start=(tf == 0),
                            stop=(tf == 7 and not kv_bias))
                    if kv_bias:
                        nc.tensor.matmul(kps, o1, bk_sb,
                                         start=False, stop=True)
                    if pend is not None:
                        emit_kvt(*pend)
                    ek = ekpool.tile([128, DLOC], bf16, tag="ek")
                    nc.scalar.activation(ek, kps, Exp)
                    sk = smpool.tile([128, HLOC], f32, tag="sk")
                    nc.vector.reduce_sum(
                        sk, ek.rearrange("p (h e) -> p h e", e=HD), axis=X)
                    rk = smpool.tile([128, HLOC], f32, tag="rk")
                    nc.vector.reciprocal(rk, sk)

                    vps = pkvp.tile([128, DLOC], f32, tag="pkv")
                    for tf in range(8):
                        nc.tensor.matmul(
                            vps, xs[:, tf, :], wv_sb[:, tf, :],
                            start=(tf == 0),
                            stop=(tf == 7 and not kv_bias))
                    if kv_bias:
                        nc.tensor.matmul(vps, o1, bv_sb,
                                         start=False, stop=True)
                    vn = vnpool.tile([128, DLOC], bf16, tag="vn")
                    rkb = bass.AP(
                        tensor=rk.tensor, offset=rk.offset,
                        ap=[list(rk.ap[0]), [1, HLOC], [0, HD]])
                    with nc.allow_low_precision(reason="bf16 ok"):
                        nc.vector.tensor_tensor(
                            out=vn.rearrange("p (h e) -> p h e", e=HD),
                            in0=vps.rearrange("p (h e) -> p h e", e=HD),
                            in1=rkb, op=mybir.AluOpType.mult)
                    pend = (st, ek, vn)
                emit_kvt(*pend)

            # ---------------- pass B: KVt -> SBUF, M = KVt @ Wo ----------
            # zero cross-head 64x64 blocks so each 128x128 pair block is
            # exactly block-diagonal, then use whole blocks as lhsT
            for p_ in range(NPAIR):
                nc.vector.memset(
                    kvt[0:64, p_ * 128 + 64:p_ * 128 + 128], 0.0)
                nc.vector.memset(
                    kvt[64:128, p_ * 128:p_ * 128 + 64], 0.0)
            nc.scalar.copy(kvt_sb, kvt)
            if dbg:
                nc.sync.dma_start(out=d_kvt, in_=kvt)
            kvctx.close()

            def emit_m_mms(mpsp):
                for p_ in range(NPAIR):
                    for jh in range(2):
                        mps = mpsp.tile([128, 512], f32, tag="mps")
                        nc.tensor.matmul(
                            mps, kvt_sb[:, p_ * 128:(p_ + 1) * 128],
                            wo_sb[:, p_, jh * 512:(jh + 1) * 512],
                            start=True, stop=True)
                        nc.scalar.copy(
                            m_sb[:, p_, jh * 512:(jh + 1) * 512], mps)
                if dbg:
                    nc.sync.dma_start(out=d_m, in_=m_sb)

            # ---------------- pass C: Q -> eqn -> y ----------------
            with ExitStack() as pC:
                eqpool = pC.enter_context(tc.tile_pool(name="eq", bufs=4))
                rqpool = pC.enter_context(tc.tile_pool(name="rq", bufs=4))
                eqnpool = pC.enter_context(tc.tile_pool(name="eqn", bufs=8))
                ysbpool = pC.enter_context(tc.tile_pool(name="ysb", bufs=3))
                qpsp = pC.enter_context(
                    tc.tile_pool(name="qps", bufs=2, space="PSUM"))
                sqpsp = pC.enter_context(
                    tc.tile_pool(name="sqps", bufs=2, space="PSUM"))
                mpsp = pC.enter_context(
                    tc.tile_pool(name="mps", bufs=2, space="PSUM"))
                ypsp = pC.enter_context(
                    tc.tile_pool(name="yps", bufs=2, space="PSUM"))

                eqn_c = {}

                def emit_c_q(c):
                    """Q-projection + exp + softmax-normalize for chunk c.

                    The sq matmul for pair dt is emitted after pair dt+1's
                    Q matmuls so the ACT exp of dt resolves while the PE
                    runs dt+1 (in-order engine streams)."""
                    eqs = []

                    def emit_sq(dt_, eq):
                        sqps = sqpsp.tile([128, SC], f32, tag="sq")
                        nc.tensor.matmul(sqps, lo_sb, eq,
                                         start=True, stop=True)
                        rq = rqpool.tile([128, SC], f32, tag="rq")
                        with nc.allow_low_precision(reason="softmax recip"):
                            nc.vector.reciprocal(rq, sqps)
                        eqn = eqnpool.tile([128, SC], bf16, tag="eqn")
                        with nc.allow_low_precision(reason="bf16 ok"):
                            nc.vector.tensor_mul(eqn, eq, rq)
                        if dbg:
                            nc.sync.dma_start(
                                out=d_eqn[:, dt_, c * SC:(c + 1) * SC],
                                in_=eqn)
                        return eqn

                    eqns = []
                    for dt_ in range(NPAIR):
                        qps = qpsp.tile([128, SC], f32, tag="q")
                        for tf in range(8):
                            nc.tensor.matmul(
                                qps, wq_sb[:, tf, dt_ * 128:(dt_ + 1) * 128],
                                xsb[:, tf, c * SC:(c + 1) * SC],
                                start=(tf == 0), stop=(tf == 7))
                        eq = eqpool.tile([128, SC], bf16, tag="eq")
                        nc.scalar.activation(eq, qps, Exp,
                                             bias=bqT[:, dt_:dt_ + 1],
                                             scale=1.0)
                        eqs.append(eq)
                        if dt_ >= 1:
                            eqns.append(emit_sq(dt_ - 1, eqs[dt_ - 1]))
                    eqns.append(emit_sq(NPAIR - 1, eqs[NPAIR - 1]))
                    eqn_c[c] = eqns

                def emit_c_y(c):
                    """y tiles for chunk c from eqn and M."""
                    eqns = eqn_c.pop(c)
                    for t in range(4):
                        ysb = ysbpool.tile([128, 2, 512], f32, tag="ysb")
                        for jh in range(2):
                            yps = ypsp.tile([128, 512], f32, tag="yps")
                            for ct in range(NPAIR):
                                nc.tensor.matmul(
                                    yps,
                                    eqns[ct][:, t * 128:(t + 1) * 128],
                                    m_sb[:, ct, jh * 512:(jh + 1) * 512],
                                    start=(ct == 0), stop=(ct == NPAIR - 1))
                            nc.scalar.copy(ysb[:, jh, :], yps)
                        row = (c * 4 + t) * 128
                        nc.sync.dma_start(
                            out=y[row:row + 128, :].rearrange(
                                "p (a b) -> p a b", a=2),
                            in_=ysb)

                # emission order keeps PE fed across the A->B->C boundary:
                # B's matmuls slot in after chunk 0's Q work (their psum
                # copy deps resolve while the Q mms run).
                emit_c_q(0)
                emit_m_mms(mpsp)
                emit_c_q(1)
                emit_c_y(0)
                for c in range(2, NCH):
                    emit_c_q(c)
                    emit_c_y(c - 1)
                emit_c_y(NCH - 1)

    with tile.TileContext(nc) as tc:
        if repeat == 1:
            body(tc)
        else:
            for _ in range(repeat):
                body(tc)
    nc.compile()
    return nc


def shard_inputs(x, Wq, bq, Wk, bk, Wv, bv, Wo, S_=S, kv_bias=False):
    import ml_dtypes
    bf = ml_dtypes.bfloat16
    L, ones1 = make_consts()
    in_maps = []
    for core in range(NCORES):
        b, g = core // GROUPS, core % GROUPS
        sl = slice(g * DLOC, (g + 1) * DLOC)
        m = {
            "xT": np.ascontiguousarray(
                np.asarray(x)[b, :S_, :].T).astype(bf),
            "wq": np.ascontiguousarray(np.asarray(Wq)[:, sl]).astype(bf),
            "wk": np.ascontiguousarray(np.asarray(Wk)[:, sl]).astype(bf),
            "wv": np.ascontiguousarray(np.asarray(Wv)[:, sl]).astype(bf),
            "wo": np.ascontiguousarray(np.asarray(Wo)[sl, :]).astype(bf),
            "bq": np.asarray(bq)[sl].astype(np.float32),
            "lones": L.astype(bf),
        }
        if kv_bias:
            m["ones1"] = ones1.astype(bf)
            m["bk"] = np.asarray(bk)[sl].astype(bf)[None, :]
            m["bv"] = np.asarray(bv)[sl].astype(bf)[None, :]
        in_maps.append(m)
    return in_maps


_NC_CACHE = {}


def _get_nc(kv_bias=False):
    key = ("nc", kv_bias)
    if key not in _NC_CACHE:
        _NC_CACHE[key] = build_bass(kv_bias=kv_bias)
    return _NC_CACHE[key]


def kernel(x, Wq, bq, Wk, bk, Wv, bv, Wo, bo):
    from concourse.bass_utils import run_bass_kernel_spmd
    kv_bias = bool(np.any(np.asarray(bk)) or np.any(np.asarray(bv)))
    nc = _get_nc(kv_bias)
    in_maps = shard_inputs(x, Wq, bq, Wk, bk, Wv, bv, Wo, kv_bias=kv_bias)
    res = run_bass_kernel_spmd(nc, in_maps, list(range(NCORES)))
    parts = [res.results[i]["y"] for i in range(NCORES)]
    out = np.stack([parts[2 * b] + parts[2 * b + 1] for b in range(B)])
    out += np.asarray(bo, dtype=np.float32)
    return out.astype(np.float32)


def oracle_core(inp, S_=S):
    """Numpy mirror of the per-core computation, for debugging."""
    xT = inp["xT"].astype(np.float64)
    Q = xT.T @ inp["wq"].astype(np.float64) + inp["bq"]
    K = xT.T @ inp["wk"].astype(np.float64)
    V = xT.T @ inp["wv"].astype(np.float64)
    out = np.zeros((S_, DLOC))
    for h in range(HLOC):
        sl = slice(h * HD, (h + 1) * HD)
        eq, ek = np.exp(Q[:, sl]), np.exp(K[:, sl])
        qh = eq / eq.sum(-1, keepdims=True)
        kh = ek / ek.sum(-1, keepdims=True)
        out[:, sl] = qh @ (kh.T @ V[:, sl])
    return (out @ inp["wo"].astype(np.float64)).astype(np.float32)


# revision 23
# speedup vs baseline: 1.4150x; 1.0767x over previous
"""Linear-attention Trainium2 kernel (8 NeuronCores, SPMD), v2.

Sharding: batch (4) x head-group (2). Core i handles batch i//2, heads
[8*(i%2), 8*(i%2)+8). Each core computes its partial y through Wo; the
host sums the two partials per batch and adds bo.

Math restructure vs v1: y = diag(recq) . expQ @ M with M = KV @ Wo_g,
so the per-head out-matmuls and all recq broadcast DMAs disappear; the
Q-softmax normalization is a block-diag-ones matmul (replicated per-head
column sums) + DVE reciprocal + DVE multiply on expQT.

All matmul operands are bf16 (same PE rate as f32r in the cost model,
half the DMA bytes); PSUM accumulation stays f32. End-to-end rel-err vs
the f32 reference is ~4e-3 (gate 2e-2).

Per-core dataflow:
  pass A (per 512-chunk c):  x chunk -> resident SBUF
    K = x_c @ Wk_g          (PE, 8 contract steps per 128-seq tile)
    ek = exp(K)             (ACT -> bf16)
    rk = 1/rowsum_h(ek)     (DVE reduce + reciprocal)
    V = x_c @ Wv_g          (PE)
    vn = V * rk             (DVE, per-head broadcast)
    KVt += vn^T-contract ek (PE, 4 pair-blocks into ONE psum bank)
  pass B: zero off-diag 64x64 blocks, KVt -> SBUF bf16,
    M_p = KVt_p-block^T-contract Wo rows  (8 mms), M -> SBUF bf16
  pass C (per chunk c):
    Q^T = Wq_g^T-contract x_c (PE), eq = exp(Q^T + bq) (ACT -> bf16)
    sq = Lones^T-contract eq  (PE, replicated per-head sums)
    eqn = eq * (1/sq)         (DVE recip + mult -> bf16)
    y_tile = eqn^T-contract M (PE psum) -> SBUF f32 -> DRAM
"""

import numpy as np

B, S, DM, H = 4, 4096, 1024, 16
HD = 64
GROUPS = 2
DLOC = DM // GROUPS   # 512 channels per core
HLOC = H // GROUPS    # 8 heads per core
NCORES = B * GROUPS   # 8
SC = 512              # sequence chunk
NPAIR = DLOC // 128   # 4 head-pair tiles


def make_consts():
    # block-diag ones [128,128]: L[d,p]=1 iff d,p in same 64-block
    L = np.zeros((128, 128), np.float32)
    L[:64, :64] = 1.0
    L[64:, 64:] = 1.0
    ones1 = np.ones((1, 128), np.float32)
    return L, ones1


def build_bass(S_=S, n_devices=NCORES, repeat=1, dbg=False, kv_bias=False):
    from contextlib import ExitStack
    import concourse.bass as bass
    import concourse.bacc as bacc
    import concourse.mybir as mybir
    import concourse.tile as tile

    f32 = mybir.dt.float32
    bf16 = mybir.dt.bfloat16
    Exp = mybir.ActivationFunctionType.Exp
    X = mybir.AxisListType.X

    NCH = S_ // SC        # sequence chunks
    NST = S_ // 128       # 128-row sequence tiles

    nc = bacc.Bacc("TRN2", target_bir_lowering=False, debug=False,
                   num_devices=n_devices)
    xT = nc.dram_tensor("xT", [DM, S_], bf16, kind="ExternalInput").ap()
    wq = nc.dram_tensor("wq", [DM, DLOC], bf16, kind="ExternalInput").ap()
    wk = nc.dram_tensor("wk", [DM, DLOC], bf16, kind="ExternalInput").ap()
    wv = nc.dram_tensor("wv", [DM, DLOC], bf16, kind="ExternalInput").ap()
    wo = nc.dram_tensor("wo", [DLOC, DM], bf16, kind="ExternalInput").ap()
    bq = nc.dram_tensor("bq", [DLOC], f32, kind="ExternalInput").ap()
    lones = nc.dram_tensor("lones", [128, 128], bf16,
                           kind="ExternalInput").ap()
    if kv_bias:
        ones1 = nc.dram_tensor("ones1", [1, 128], bf16,
                               kind="ExternalInput").ap()
        bk = nc.dram_tensor("bk", [1, DLOC], bf16, kind="ExternalInput").ap()
        bv = nc.dram_tensor("bv", [1, DLOC], bf16, kind="ExternalInput").ap()
    y = nc.dram_tensor("y", [S_, DM], f32, kind="ExternalOutput").ap()
    if dbg:
        d_eqn = nc.dram_tensor("d_eqn", [128, NPAIR, S_], bf16,
                               kind="ExternalOutput").ap()
        d_kvt = nc.dram_tensor("d_kvt", [128, 512], bf16,
                               kind="ExternalOutput").ap()
        d_m = nc.dram_tensor("d_m", [128, NPAIR, DM], bf16,
                             kind="ExternalOutput").ap()

    xTr = xT.rearrange("(tf p) s -> p tf s", p=128)

    def body(tc):
        ctx = ExitStack()
        with ctx:
            cons = ctx.enter_context(tc.tile_pool(name="cons", bufs=1))
            persist = ctx.enter_context(tc.tile_pool(name="persist", bufs=1))

            # DMA issue order is warm-up critical: the DMA data transfers
            # serialize, so interleave weight loads with the first x chunks
            # (wk -> x[st 0:2] -> wv -> x[st 2:4] -> x[c1] -> wq -> ...) and
            # push pass-B/C-only loads behind the early x chunks.
            bqT = cons.tile([128, NPAIR], f32)
            lo_sb = cons.tile([128, 128], bf16)
            if kv_bias:
                o1 = cons.tile([1, 128], bf16)
                bk_sb = cons.tile([1, DLOC], bf16)
                bv_sb = cons.tile([1, DLOC], bf16)

            wk_sb = persist.tile([128, 8, DLOC], bf16, tag="wk")
            wv_sb = persist.tile([128, 8, DLOC], bf16, tag="wv")
            wq_sb = persist.tile([128, 8, DLOC], bf16, tag="wq")
            wo_sb = persist.tile([128, NPAIR, DM], bf16, tag="wo")
            xsb = persist.tile([128, 8, S_], bf16, tag="xsb")

            def emit_dma_weights(which):
                if which == "wk":
                    nc.sync.dma_start(
                        out=wk_sb,
                        in_=wk.rearrange("(tf p) d -> p tf d", p=128))
                    if kv_bias:
                        nc.sync.dma_start(out=o1, in_=ones1)
                        nc.sync.dma_start(out=bk_sb, in_=bk)
                elif which == "wv":
                    nc.sync.dma_start(
                        out=wv_sb,
                        in_=wv.rearrange("(tf p) d -> p tf d", p=128))
                    if kv_bias:
                        nc.sync.dma_start(out=bv_sb, in_=bv)
                elif which == "wq":
                    nc.sync.dma_start(
                        out=wq_sb,
                        in_=wq.rearrange("(tf p) d -> p tf d", p=128))
                    nc.sync.dma_start(
                        out=bqT, in_=bq.rearrange("(t p) -> p t", p=128))
                    nc.sync.dma_start(out=lo_sb, in_=lones)
                elif which == "wo":
                    nc.sync.dma_start(
                        out=wo_sb,
                        in_=wo.rearrange("(t p) j -> p t j", p=128))
            m_sb = persist.tile([128, NPAIR, DM], bf16, tag="msb")
            kvt_sb = persist.tile([128, 512], bf16, tag="kvtsb")

            kvctx = ExitStack()
            kvpsp = kvctx.enter_context(
                tc.tile_pool(name="kvps", bufs=1, space="PSUM"))
            kvt = kvpsp.tile([128, 512], f32, tag="kvt")

            # ---------------- pass A: K/V -> KVt ----------------
            with ExitStack() as pA:
                ekpool = pA.enter_context(tc.tile_pool(name="ek", bufs=4))
                vnpool = pA.enter_context(tc.tile_pool(name="vn", bufs=4))
                smpool = pA.enter_context(tc.tile_pool(name="sm", bufs=4))
                pkvp = pA.enter_context(
                    tc.tile_pool(name="pkv", bufs=4, space="PSUM"))

                # KVt[e, d] += vn_h^T @ ek_h per head into its diagonal
                # 64x64 sub-block; off-diagonal blocks are never written so
                # the start=True full-row clears (heads 0/1, st=0) leave
                # them zero and the pair blocks are exactly block-diagonal.
                # Emitted one seq-tile late so the ek/vn ACT+DVE chain of
                # tile st resolves while tile st+1's K matmuls run.
                def emit_kvt(st, ek, vn):
                    for h in range(HLOC):
                        p_, sub = h // 2, h % 2
                        rows = slice(64 * sub, 64 * sub + 64)
                        cols = slice(p_ * 128 + 64 * sub,
                                     p_ * 128 + 64 * sub + 64)
                        nc.tensor.matmul(
                            kvt[rows, cols],
                            vn[:, h * HD:(h + 1) * HD],
                            ek[:, h * HD:(h + 1) * HD],
                            start=(st == 0 and h < 2),
                            stop=(st == NST - 1),
                            skip_group_check=True)

                emit_dma_weights("wk")
                pend = None
                for st in range(NST):
                    if st == 0:
                        # first chunk in two halves so K matmuls start as
                        # early as possible
                        nc.sync.dma_start(
                            out=xsb[:, :, 0:256], in_=xTr[:, :, 0:256])
                        emit_dma_weights("wv")
                        nc.sync.dma_start(
                            out=xsb[:, :, 256:512], in_=xTr[:, :, 256:512])
                    elif st % 4 == 0:
                        c = st // 4
                        nc.sync.dma_start(
                            out=xsb[:, :, c * SC:(c + 1) * SC],
                            in_=xTr[:, :, c * SC:(c + 1) * SC])
                        if c == 1:
                            emit_dma_weights("wq")
                        elif c == 2:
                            emit_dma_weights("wo")
                    xs = xsb[:, :, st * 128:(st + 1) * 128]
                    kps = pkvp.tile([128, DLOC], f32, tag="pkv")
                    for tf in range(8):
                        nc.tensor.matmul(
                            kps, xs[:, tf, :], wk_sb[:, tf, :],
                            start=(tf == 0),
                            stop=(tf == 7 and not kv_bias))
                    if kv_bias:
                        nc.tensor.matmul(kps, o1, bk_sb,
                                         start=False, stop=True)
                    if pend is not None:
                        emit_kvt(*pend)
                    ek = ekpool.tile([128, DLOC], bf16, tag="ek")
                    nc.scalar.activation(ek, kps, Exp)
                    sk = smpool.tile([128, HLOC], f32, tag="sk")
                    nc.vector.reduce_sum(
                        sk, ek.rearrange("p (h e) -> p h e", e=HD), axis=X)
                    rk = smpool.tile([128, HLOC], f32, tag="rk")
                    nc.vector.reciprocal(rk, sk)

                    vps = pkvp.tile([128, DLOC], f32, tag="pkv")
                    for tf in range(8):
                        nc.tensor.matmul(
                            vps, xs[:, tf, :], wv_sb[:, tf, :],
                            start=(tf == 0),
                            stop=(tf == 7 and not kv_bias))
                    if kv_bias:
                        nc.tensor.matmul(vps, o1, bv_sb,
                                         start=False, stop=True)
                    vn = vnpool.tile([128, DLOC], bf16, tag="vn")
                    rkb = bass.AP(
                        tensor=rk.tensor, offset=rk.offset,
                        ap=[list(rk.ap[0]), [1, HLOC], [0, HD]])
                    with nc.allow_low_precision(reason="bf16 ok"):
                        nc.vector.tensor_tensor(
                            out=vn.rearrange("p (h e) -> p h e", e=HD),
                            in0=vps.rearrange("p (h e) -> p h e", e=HD),
                            in1=rkb, op=mybir.AluOpType.mult)
                    pend = (st, ek, vn)
                emit_kvt(*pend)

            # ---------------- pass B: KVt -> SBUF, M = KVt @ Wo ----------
            nc.scalar.copy(kvt_sb, kvt)
            kvctx.close()

            def emit_m_mms(mpsp):
                if dbg:
                    nc.sync.dma_start(out=d_kvt, in_=kvt_sb)
                for p_ in range(NPAIR):
                    for jh in range(2):
                        mps = mpsp.tile([128, 512], f32, tag="mps")
                        nc.tensor.matmul(
                            mps, kvt_sb[:, p_ * 128:(p_ + 1) * 128],
                            wo_sb[:, p_, jh * 512:(jh + 1) * 512],
                            start=True, stop=True)
                        nc.scalar.copy(
                            m_sb[:, p_, jh * 512:(jh + 1) * 512], mps)
                if dbg:
                    nc.sync.dma_start(out=d_m, in_=m_sb)

            # ---------------- pass C: Q -> eqn -> y ----------------
            with ExitStack() as pC:
                eqpool = pC.enter_context(tc.tile_pool(name="eq", bufs=4))
                rqpool = pC.enter_context(tc.tile_pool(name="rq", bufs=4))
                eqnpool = pC.enter_context(tc.tile_pool(name="eqn", bufs=8))
                ysbpool = pC.enter_context(tc.tile_pool(name="ysb", bufs=4))
                qpsp = pC.enter_context(
                    tc.tile_pool(name="qps", bufs=2, space="PSUM"))
                sqpsp = pC.enter_context(
                    tc.tile_pool(name="sqps", bufs=1, space="PSUM"))
                mpsp = pC.enter_context(
                    tc.tile_pool(name="mps", bufs=2, space="PSUM"))
                ypsp = pC.enter_context(
                    tc.tile_pool(name="yps", bufs=3, space="PSUM"))

                eqn_c = {}

                def emit_c_q(c):
                    """Q-projection + exp + softmax-normalize for chunk c.

                    The sq matmul for pair dt is emitted after pair dt+1's
                    Q matmuls so the ACT exp of dt resolves while the PE
                    runs dt+1 (in-order engine streams)."""
                    eqs = []

                    def emit_sq(dt_, eq):
                        sqps = sqpsp.tile([128, SC], f32, tag="sq")
                        nc.tensor.matmul(sqps, lo_sb, eq,
                                         start=True, stop=True)
                        rq = rqpool.tile([128, SC], bf16, tag="rq")
                        with nc.allow_low_precision(reason="softmax recip"):
                            nc.vector.reciprocal(rq, sqps)
                        eqn = eqnpool.tile([128, SC], bf16, tag="eqn")
                        with nc.allow_low_precision(reason="bf16 ok"):
                            nc.vector.tensor_mul(eqn, eq, rq)
                        if dbg:
                            nc.sync.dma_start(
                                out=d_eqn[:, dt_, c * SC:(c + 1) * SC],
                                in_=eqn)
                        return eqn

                    eqns = []
                    for dt_ in range(NPAIR):
                        qps = qpsp.tile([128, SC], f32, tag="q")
                        for tf in range(8):
                            nc.tensor.matmul(
                                qps, wq_sb[:, tf, dt_ * 128:(dt_ + 1) * 128],
                                xsb[:, tf, c * SC:(c + 1) * SC],
                                start=(tf == 0), stop=(tf == 7))
                        eq = eqpool.tile([128, SC], bf16, tag="eq")
                        nc.scalar.activation(eq, qps, Exp,
                                             bias=bqT[:, dt_:dt_ + 1],
                                             scale=1.0)
                        eqs.append(eq)
                        if dt_ >= 1:
                            eqns.append(emit_sq(dt_ - 1, eqs[dt_ - 1]))
                    eqns.append(emit_sq(NPAIR - 1, eqs[NPAIR - 1]))
                    eqn_c[c] = eqns

                def emit_c_y(c):
                    """y tiles for chunk c from eqn and M. Copy each jh
                    half to SBUF and DMA it immediately (shorter drain
                    than one copy+DMA per full tile)."""
                    eqns = eqn_c.pop(c)
                    for t in range(4):
                        row = (c * 4 + t) * 128
                        for jh in range(2):
                            yps = ypsp.tile([128, 512], f32, tag="yps")
                            for ct in range(NPAIR):
                                nc.tensor.matmul(
                                    yps,
                                    eqns[ct][:, t * 128:(t + 1) * 128],
                                    m_sb[:, ct, jh * 512:(jh + 1) * 512],
                                    start=(ct == 0), stop=(ct == NPAIR - 1))
                            ysb = ysbpool.tile([128, 512], f32, tag="ysb")
                            nc.scalar.copy(ysb, yps)
                            nc.sync.dma_start(
                                out=y[row:row + 128,
                                      jh * 512:(jh + 1) * 512],
                                in_=ysb)

                # emission order keeps PE fed across the A->B->C boundary:
                # B's matmuls slot in after chunk 0's Q work (their psum
                # copy deps resolve while the Q mms run).
                emit_c_q(0)
                emit_m_mms(mpsp)
                emit_c_q(1)
                emit_c_y(0)
                for c in range(2, NCH):
                    emit_c_q(c)
                    emit_c_y(c - 1)
                emit_c_y(NCH - 1)

    with tile.TileContext(nc) as tc:
        if repeat == 1:
            body(tc)
        else:
            for _ in range(repeat):
                body(tc)
    nc.compile()
    return nc


def shard_inputs(x, Wq, bq, Wk, bk, Wv, bv, Wo, S_=S, kv_bias=False):
    import ml_dtypes
    bf = ml_dtypes.bfloat16
    L, ones1 = make_consts()
    in_maps = []
    for core in range(NCORES):
        b, g = core // GROUPS, core % GROUPS
        sl = slice(g * DLOC, (g + 1) * DLOC)
        m = {
            "xT": np.ascontiguousarray(
                np.asarray(x)[b, :S_, :].T).astype(bf),
            "wq": np.ascontiguousarray(np.asarray(Wq)[:, sl]).astype(bf),
            "wk": np.ascontiguousarray(np.asarray(Wk)[:, sl]).astype(bf),
            "wv": np.ascontiguousarray(np.asarray(Wv)[:, sl]).astype(bf),
            "wo": np.ascontiguousarray(np.asarray(Wo)[sl, :]).astype(bf),
            "bq": np.asarray(bq)[sl].astype(np.float32),
            "lones": L.astype(bf),
        }
        if kv_bias:
            m["ones1"] = ones1.astype(bf)
            m["bk"] = np.asarray(bk)[sl].astype(bf)[None, :]
            m["bv"] = np.asarray(bv)[sl].astype(bf)[None, :]
        in_maps.append(m)
    return in_maps


_NC_CACHE = {}


def _get_nc(kv_bias=False):
    key = ("nc", kv_bias)
    if key not in _NC_CACHE:
        _NC_CACHE[key] = build_bass(kv_bias=kv_bias)
    return _NC_CACHE[key]


def kernel(x, Wq, bq, Wk, bk, Wv, bv, Wo, bo):
    from concourse.bass_utils import run_bass_kernel_spmd
    kv_bias = bool(np.any(np.asarray(bk)) or np.any(np.asarray(bv)))
    nc = _get_nc(kv_bias)
    in_maps = shard_inputs(x, Wq, bq, Wk, bk, Wv, bv, Wo, kv_bias=kv_bias)
    res = run_bass_kernel_spmd(nc, in_maps, list(range(NCORES)))
    parts = [res.results[i]["y"] for i in range(NCORES)]
    out = np.stack([parts[2 * b] + parts[2 * b + 1] for b in range(B)])
    out += np.asarray(bo, dtype=np.float32)
    return out.astype(np.float32)


def oracle_core(inp, S_=S):
    """Numpy mirror of the per-core computation, for debugging."""
    xT = inp["xT"].astype(np.float64)
    Q = xT.T @ inp["wq"].astype(np.float64) + inp["bq"]
    K = xT.T @ inp["wk"].astype(np.float64)
    V = xT.T @ inp["wv"].astype(np.float64)
    out = np.zeros((S_, DLOC))
    for h in range(HLOC):
        sl = slice(h * HD, (h + 1) * HD)
        eq, ek = np.exp(Q[:, sl]), np.exp(K[:, sl])
        qh = eq / eq.sum(-1, keepdims=True)
        kh = ek / ek.sum(-1, keepdims=True)
        out[:, sl] = qh @ (kh.T @ V[:, sl])
    return (out @ inp["wo"].astype(np.float64)).astype(np.float32)


# revision 31
# speedup vs baseline: 1.4471x; 1.0227x over previous
"""Linear-attention Trainium2 kernel (8 NeuronCores, SPMD), v2.

Sharding: batch (4) x head-group (2). Core i handles batch i//2, heads
[8*(i%2), 8*(i%2)+8). Each core computes its partial y through Wo; the
host sums the two partials per batch and adds bo.

Math restructure vs v1: y = diag(recq) . expQ @ M with M = KV @ Wo_g,
so the per-head out-matmuls and all recq broadcast DMAs disappear; the
Q-softmax normalization is a block-diag-ones matmul (replicated per-head
column sums) + DVE reciprocal + DVE multiply on expQT.

All matmul operands are bf16 (same PE rate as f32r in the cost model,
half the DMA bytes); PSUM accumulation stays f32. End-to-end rel-err vs
the f32 reference is ~4e-3 (gate 2e-2).

Per-core dataflow:
  pass A (per 512-chunk c):  x chunk -> resident SBUF
    K = x_c @ Wk_g          (PE, 8 contract steps per 128-seq tile)
    ek = exp(K)             (ACT -> bf16)
    rk = 1/rowsum_h(ek)     (DVE reduce + reciprocal)
    V = x_c @ Wv_g          (PE)
    vn = V * rk             (DVE, per-head broadcast)
    KVt += vn^T-contract ek (PE, 4 pair-blocks into ONE psum bank)
  pass B: zero off-diag 64x64 blocks, KVt -> SBUF bf16,
    M_p = KVt_p-block^T-contract Wo rows  (8 mms), M -> SBUF bf16
  pass C (per chunk c):
    Q^T = Wq_g^T-contract x_c (PE), eq = exp(Q^T + bq) (ACT -> bf16)
    sq = Lones^T-contract eq  (PE, replicated per-head sums)
    eqn = eq * (1/sq)         (DVE recip + mult -> bf16)
    y_tile = eqn^T-contract M (PE psum) -> SBUF f32 -> DRAM
"""

import numpy as np

B, S, DM, H = 4, 4096, 1024, 16
HD = 64
GROUPS = 2
DLOC = DM // GROUPS   # 512 channels per core
HLOC = H // GROUPS    # 8 heads per core
NCORES = B * GROUPS   # 8
SC = 512              # sequence chunk
NPAIR = DLOC // 128   # 4 head-pair tiles


def make_consts():
    # block-diag ones [128,128]: L[d,p]=1 iff d,p in same 64-block
    L = np.zeros((128, 128), np.float32)
    L[:64, :64] = 1.0
    L[64:, 64:] = 1.0
    ones1 = np.ones((1, 128), np.float32)
    return L, ones1


def build_bass(S_=S, n_devices=NCORES, repeat=1, dbg=False, kv_bias=False):
    from contextlib import ExitStack
    import concourse.bass as bass
    import concourse.bacc as bacc
    import concourse.mybir as mybir
    import concourse.tile as tile

    f32 = mybir.dt.float32
    bf16 = mybir.dt.bfloat16
    Exp = mybir.ActivationFunctionType.Exp
    X = mybir.AxisListType.X

    NCH = S_ // SC        # sequence chunks
    NST = S_ // 128       # 128-row sequence tiles

    nc = bacc.Bacc("TRN2", target_bir_lowering=False, debug=False,
                   num_devices=n_devices)
    xT = nc.dram_tensor("xT", [DM, S_], bf16, kind="ExternalInput").ap()
    wq = nc.dram_tensor("wq", [DM, DLOC], bf16, kind="ExternalInput").ap()
    wk = nc.dram_tensor("wk", [DM, DLOC], bf16, kind="ExternalInput").ap()
    wv = nc.dram_tensor("wv", [DM, DLOC], bf16, kind="ExternalInput").ap()
    wo = nc.dram_tensor("wo", [DLOC, DM], bf16, kind="ExternalInput").ap()
    bq = nc.dram_tensor("bq", [DLOC], f32, kind="ExternalInput").ap()
    lones = nc.dram_tensor("lones", [128, 128], bf16,
                           kind="ExternalInput").ap()
    if kv_bias:
        ones1 = nc.dram_tensor("ones1", [1, 128], bf16,
                               kind="ExternalInput").ap()
        bk = nc.dram_tensor("bk", [1, DLOC], bf16, kind="ExternalInput").ap()
        bv = nc.dram_tensor("bv", [1, DLOC], bf16, kind="ExternalInput").ap()
    y = nc.dram_tensor("y", [S_, DM], f32, kind="ExternalOutput").ap()
    if dbg:
        d_eqn = nc.dram_tensor("d_eqn", [128, NPAIR, S_], bf16,
                               kind="ExternalOutput").ap()
        d_kvt = nc.dram_tensor("d_kvt", [128, 512], bf16,
                               kind="ExternalOutput").ap()
        d_m = nc.dram_tensor("d_m", [128, NPAIR, DM], bf16,
                             kind="ExternalOutput").ap()

    xTr = xT.rearrange("(tf p) s -> p tf s", p=128)

    def body(tc):
        ctx = ExitStack()
        with ctx:
            cons = ctx.enter_context(tc.tile_pool(name="cons", bufs=1))
            persist = ctx.enter_context(tc.tile_pool(name="persist", bufs=1))

            # DMA issue order is warm-up critical: the DMA data transfers
            # serialize, so interleave weight loads with the first x chunks
            # (wk -> x[st 0:2] -> wv -> x[st 2:4] -> x[c1] -> wq -> ...) and
            # push pass-B/C-only loads behind the early x chunks.
            bqT = cons.tile([128, NPAIR], f32)
            lo_sb = cons.tile([128, 128], bf16)
            if kv_bias:
                o1 = cons.tile([1, 128], bf16)
                bk_sb = cons.tile([1, DLOC], bf16)
                bv_sb = cons.tile([1, DLOC], bf16)

            wk_sb = persist.tile([128, 8, DLOC], bf16, tag="wk")
            wv_sb = persist.tile([128, 8, DLOC], bf16, tag="wv")
            wq_sb = persist.tile([128, 8, DLOC], bf16, tag="wq")
            wo_sb = persist.tile([128, NPAIR, DM], bf16, tag="wo")
            xsb = persist.tile([128, 8, S_], bf16, tag="xsb")

            wk_r = wk.rearrange("(tf p) d -> p tf d", p=128)

            def emit_dma_weights(which):
                if which == "wk":
                    nc.sync.dma_start(out=wk_sb[:, 0:4, :], in_=wk_r[:, 0:4])
                    if kv_bias:
                        nc.sync.dma_start(out=o1, in_=ones1)
                        nc.sync.dma_start(out=bk_sb, in_=bk)
                elif which == "wk2":
                    nc.sync.dma_start(out=wk_sb[:, 4:8, :], in_=wk_r[:, 4:8])
                elif which == "wv":
                    nc.sync.dma_start(
                        out=wv_sb,
                        in_=wv.rearrange("(tf p) d -> p tf d", p=128))
                    if kv_bias:
                        nc.sync.dma_start(out=bv_sb, in_=bv)
                elif which == "wq":
                    nc.sync.dma_start(
                        out=wq_sb,
                        in_=wq.rearrange("(tf p) d -> p tf d", p=128))
                    nc.sync.dma_start(
                        out=bqT, in_=bq.rearrange("(t p) -> p t", p=128))
                    nc.sync.dma_start(out=lo_sb, in_=lones)
                elif which == "wo":
                    nc.sync.dma_start(
                        out=wo_sb,
                        in_=wo.rearrange("(t p) j -> p t j", p=128))
            m_sb = persist.tile([128, NPAIR, DM], bf16, tag="msb")
            kvt_sb = persist.tile([128, 512], bf16, tag="kvtsb")

            # pass C pools open for the whole kernel so chunk-0 Q work can
            # be emitted inside pass A (fills the PE gap while the last
            # seq-tile's ek/vn chain resolves).
            eqpool = ctx.enter_context(tc.tile_pool(name="eq", bufs=4))
            rqpool = ctx.enter_context(tc.tile_pool(name="rq", bufs=4))
            eqnpool = ctx.enter_context(tc.tile_pool(name="eqn", bufs=8))
            ysbpool = ctx.enter_context(tc.tile_pool(name="ysb", bufs=4))
            qpsp = ctx.enter_context(
                tc.tile_pool(name="qps", bufs=2, space="PSUM"))
            sqpsp = ctx.enter_context(
                tc.tile_pool(name="sqps", bufs=1, space="PSUM"))

            kvctx = ExitStack()
            kvpsp = kvctx.enter_context(
                tc.tile_pool(name="kvps", bufs=1, space="PSUM"))
            kvt = kvpsp.tile([128, 512], f32, tag="kvt")
            # The KVt matmuls only ever write the per-head diagonal 64x64
            # sub-blocks; start=True clears has_written flags but NOT the
            # stale psum DATA in the off-diagonal blocks, which the
            # pass-B copy reads. Zero them up front (overlaps pass A).
            for p_ in range(NPAIR):
                nc.vector.memset(
                    kvt[0:64, p_ * 128 + 64:p_ * 128 + 128], 0.0)
                nc.vector.memset(
                    kvt[64:128, p_ * 128:p_ * 128 + 64], 0.0)

            eqn_c = {}

            def emit_c_q(c):
                """Q-projection + exp + softmax-normalize for chunk c.

                The sq matmul for pair dt is emitted after pair dt+1's
                Q matmuls so the ACT exp of dt resolves while the PE
                runs dt+1 (in-order engine streams)."""
                eqs = []

                def emit_sq(dt_, eq):
                    sqps = sqpsp.tile([128, SC], f32, tag="sq")
                    nc.tensor.matmul(sqps, lo_sb, eq, start=True, stop=True)
                    rq = rqpool.tile([128, SC], bf16, tag="rq")
                    with nc.allow_low_precision(reason="softmax recip"):
                        nc.vector.reciprocal(rq, sqps)
                    eqn = eqnpool.tile([128, SC], bf16, tag="eqn")
                    with nc.allow_low_precision(reason="bf16 ok"):
                        nc.vector.tensor_mul(eqn, eq, rq)
                    if dbg:
                        nc.sync.dma_start(
                            out=d_eqn[:, dt_, c * SC:(c + 1) * SC], in_=eqn)
                    return eqn

                eqns = []
                for dt_ in range(NPAIR):
                    qps = qpsp.tile([128, SC], f32, tag="q")
                    for tf in range(8):
                        nc.tensor.matmul(
                            qps, wq_sb[:, tf, dt_ * 128:(dt_ + 1) * 128],
                            xsb[:, tf, c * SC:(c + 1) * SC],
                            start=(tf == 0), stop=(tf == 7))
                    eq = eqpool.tile([128, SC], bf16, tag="eq")
                    nc.scalar.activation(eq, qps, Exp,
                                         bias=bqT[:, dt_:dt_ + 1],
                                         scale=1.0)
                    eqs.append(eq)
                    if dt_ >= 1:
                        eqns.append(emit_sq(dt_ - 1, eqs[dt_ - 1]))
                eqns.append(emit_sq(NPAIR - 1, eqs[NPAIR - 1]))
                eqn_c[c] = eqns

            # ---------------- pass A: K/V -> KVt ----------------
            with ExitStack() as pA:
                ekpool = pA.enter_context(tc.tile_pool(name="ek", bufs=4))
                vnpool = pA.enter_context(tc.tile_pool(name="vn", bufs=4))
                smpool = pA.enter_context(tc.tile_pool(name="sm", bufs=4))
                pkvp = pA.enter_context(
                    tc.tile_pool(name="pkv", bufs=4, space="PSUM"))

                # KVt[e, d] += vn_h^T @ ek_h per head into its diagonal
                # 64x64 sub-block; off-diagonal blocks are never written so
                # the start=True full-row clears (heads 0/1, st=0) leave
                # them zero and the pair blocks are exactly block-diagonal.
                # Emitted one seq-tile late so the ek/vn ACT+DVE chain of
                # tile st resolves while tile st+1's K matmuls run.
                def emit_kvt(st, ek, vn):
                    for h in range(HLOC):
                        p_, sub = h // 2, h % 2
                        rows = slice(64 * sub, 64 * sub + 64)
                        cols = slice(p_ * 128 + 64 * sub,
                                     p_ * 128 + 64 * sub + 64)
                        nc.tensor.matmul(
                            kvt[rows, cols],
                            vn[:, h * HD:(h + 1) * HD],
                            ek[:, h * HD:(h + 1) * HD],
                            start=(st == 0 and h < 2),
                            stop=(st == NST - 1),
                            skip_group_check=True)

                def emit_k(st):
                    xs = xsb[:, :, st * 128:(st + 1) * 128]
                    kps = pkvp.tile([128, DLOC], f32, tag="pkv")
                    for tf in range(8):
                        nc.tensor.matmul(
                            kps, xs[:, tf, :], wk_sb[:, tf, :],
                            start=(tf == 0),
                            stop=(tf == 7 and not kv_bias))
                    if kv_bias:
                        nc.tensor.matmul(kps, o1, bk_sb,
                                         start=False, stop=True)
                    ek = ekpool.tile([128, DLOC], bf16, tag="ek")
                    nc.scalar.activation(ek, kps, Exp)
                    sk = smpool.tile([128, HLOC], f32, tag="sk")
                    nc.vector.reduce_sum(
                        sk, ek.rearrange("p (h e) -> p h e", e=HD), axis=X)
                    rk = smpool.tile([128, HLOC], f32, tag="rk")
                    nc.vector.reciprocal(rk, sk)
                    return ek, rk

                def emit_v(st, rk):
                    xs = xsb[:, :, st * 128:(st + 1) * 128]
                    vps = pkvp.tile([128, DLOC], f32, tag="pkv")
                    for tf in range(8):
                        nc.tensor.matmul(
                            vps, xs[:, tf, :], wv_sb[:, tf, :],
                            start=(tf == 0),
                            stop=(tf == 7 and not kv_bias))
                    if kv_bias:
                        nc.tensor.matmul(vps, o1, bv_sb,
                                         start=False, stop=True)
                    vn = vnpool.tile([128, DLOC], bf16, tag="vn")
                    rkb = bass.AP(
                        tensor=rk.tensor, offset=rk.offset,
                        ap=[list(rk.ap[0]), [1, HLOC], [0, HD]])
                    with nc.allow_low_precision(reason="bf16 ok"):
                        nc.vector.tensor_tensor(
                            out=vn.rearrange("p (h e) -> p h e", e=HD),
                            in0=vps.rearrange("p (h e) -> p h e", e=HD),
                            in1=rkb, op=mybir.AluOpType.mult)
                    return vn

                # PE order per step: K(st), KVt(st-2), V(st-1) — the V
                # stream trails by one tile and KVt by two, so the ACT/DVE
                # chains (ek -> rk -> vn) have a full tile of slack and the
                # startup DMAs (wk halves, x, wv) stay ahead of first use.
                emit_dma_weights("wk")
                ekrk = {}
                vnd = {}
                for st in range(NST + 1):
                    if st == 0:
                        nc.sync.dma_start(
                            out=xsb[:, :, 0:256], in_=xTr[:, :, 0:256])
                        emit_dma_weights("wk2")
                        nc.sync.dma_start(
                            out=xsb[:, :, 256:512], in_=xTr[:, :, 256:512])
                        emit_dma_weights("wv")
                    elif st % 4 == 0 and st < NST:
                        c = st // 4
                        nc.sync.dma_start(
                            out=xsb[:, :, c * SC:(c + 1) * SC],
                            in_=xTr[:, :, c * SC:(c + 1) * SC])
                        if c == 1:
                            emit_dma_weights("wq")
                        elif c == 2:
                            emit_dma_weights("wo")
                    if st < NST:
                        ekrk[st] = emit_k(st)
                    if st >= 2:
                        ek, _ = ekrk.pop(st - 2)
                        emit_kvt(st - 2, ek, vnd.pop(st - 2))
                    if st >= 1:
                        vnd[st - 1] = emit_v(st - 1, ekrk[st - 1][1])
                # chunk-0 Q work fills the PE while ek/vn of the last tile
                # resolve, then the final KVt group closes the accumulation
                emit_c_q(0)
                ek, _ = ekrk.pop(NST - 1)
                emit_kvt(NST - 1, ek, vnd.pop(NST - 1))

            # ---------------- pass B: KVt -> SBUF, M = KVt @ Wo ----------
            nc.scalar.copy(kvt_sb, kvt)
            kvctx.close()

            def emit_m_mms(mpsp):
                if dbg:
                    nc.sync.dma_start(out=d_kvt, in_=kvt_sb)
                for p_ in range(NPAIR):
                    for jh in range(2):
                        mps = mpsp.tile([128, 512], f32, tag="mps")
                        nc.tensor.matmul(
                            mps, kvt_sb[:, p_ * 128:(p_ + 1) * 128],
                            wo_sb[:, p_, jh * 512:(jh + 1) * 512],
                            start=True, stop=True)
                        nc.scalar.copy(
                            m_sb[:, p_, jh * 512:(jh + 1) * 512], mps)
                if dbg:
                    nc.sync.dma_start(out=d_m, in_=m_sb)

            # ---------------- pass C: y tiles ----------------
            with ExitStack() as pC:
                mpsp = pC.enter_context(
                    tc.tile_pool(name="mps", bufs=2, space="PSUM"))
                ypsp = pC.enter_context(
                    tc.tile_pool(name="yps", bufs=3, space="PSUM"))

                def emit_c_y(c):
                    """y tiles for chunk c from eqn and M. Copy each jh
                    half to SBUF (alternating ACT/DVE) and DMA it
                    immediately (shorter drain than one copy per tile)."""
                    eqns = eqn_c.pop(c)
                    for t in range(4):
                        row = (c * 4 + t) * 128
                        for jh in range(2):
                            yps = ypsp.tile([128, 512], f32, tag="yps")
                            for ct in range(NPAIR):
                                nc.tensor.matmul(
                                    yps,
                                    eqns[ct][:, t * 128:(t + 1) * 128],
                                    m_sb[:, ct, jh * 512:(jh + 1) * 512],
                                    start=(ct == 0), stop=(ct == NPAIR - 1))
                            ysb = ysbpool.tile([128, 512], f32, tag="ysb")
                            if jh == 0:
                                nc.scalar.copy(ysb, yps)
                            else:
                                nc.vector.tensor_scalar_mul(ysb, yps, 1.0)
                            nc.sync.dma_start(
                                out=y[row:row + 128,
                                      jh * 512:(jh + 1) * 512],
                                in_=ysb)

                # emission order keeps PE fed across the A->B->C boundary:
                # B's matmuls slot in after chunk 0's Q work (their psum
                # copy deps resolve while the Q mms run).
                emit_m_mms(mpsp)
                emit_c_q(1)
                emit_c_y(0)
                for c in range(2, NCH):
                    emit_c_q(c)
                    emit_c_y(c - 1)
                emit_c_y(NCH - 1)

    with tile.TileContext(nc) as tc:
        if repeat == 1:
            body(tc)
        else:
            for _ in range(repeat):
                body(tc)
    nc.compile()
    return nc


def shard_inputs(x, Wq, bq, Wk, bk, Wv, bv, Wo, S_=S, kv_bias=False):
    import ml_dtypes
    bf = ml_dtypes.bfloat16
    L, ones1 = make_consts()
    in_maps = []
    for core in range(NCORES):
        b, g = core // GROUPS, core % GROUPS
        sl = slice(g * DLOC, (g + 1) * DLOC)
        m = {
            "xT": np.ascontiguousarray(
                np.asarray(x)[b, :S_, :].T).astype(bf),
            "wq": np.ascontiguousarray(np.asarray(Wq)[:, sl]).astype(bf),
            "wk": np.ascontiguousarray(np.asarray(Wk)[:, sl]).astype(bf),
            "wv": np.ascontiguousarray(np.asarray(Wv)[:, sl]).astype(bf),
            "wo": np.ascontiguousarray(np.asarray(Wo)[sl, :]).astype(bf),
            "bq": np.asarray(bq)[sl].astype(np.float32),
            "lones": L.astype(bf),
        }
        if kv_bias:
            m["ones1"] = ones1.astype(bf)
            m["bk"] = np.asarray(bk)[sl].astype(bf)[None, :]
            m["bv"] = np.asarray(bv)[sl].astype(bf)[None, :]
        in_maps.append(m)
    return in_maps


_NC_CACHE = {}


def _get_nc(kv_bias=False):
    key = ("nc", kv_bias)
    if key not in _NC_CACHE:
        _NC_CACHE[key] = build_bass(kv_bias=kv_bias)
    return _NC_CACHE[key]


def kernel(x, Wq, bq, Wk, bk, Wv, bv, Wo, bo):
    from concourse.bass_utils import run_bass_kernel_spmd
    kv_bias = bool(np.any(np.asarray(bk)) or np.any(np.asarray(bv)))
    nc = _get_nc(kv_bias)
    in_maps = shard_inputs(x, Wq, bq, Wk, bk, Wv, bv, Wo, kv_bias=kv_bias)
    res = run_bass_kernel_spmd(nc, in_maps, list(range(NCORES)))
    parts = [res.results[i]["y"] for i in range(NCORES)]
    out = np.stack([parts[2 * b] + parts[2 * b + 1] for b in range(B)])
    out += np.asarray(bo, dtype=np.float32)
    return out.astype(np.float32)


def oracle_core(inp, S_=S):
    """Numpy mirror of the per-core computation, for debugging."""
    xT = inp["xT"].astype(np.float64)
    Q = xT.T @ inp["wq"].astype(np.float64) + inp["bq"]
    K = xT.T @ inp["wk"].astype(np.float64)
    V = xT.T @ inp["wv"].astype(np.float64)
    out = np.zeros((S_, DLOC))
    for h in range(HLOC):
        sl = slice(h * HD, (h + 1) * HD)
        eq, ek = np.exp(Q[:, sl]), np.exp(K[:, sl])
        qh = eq / eq.sum(-1, keepdims=True)
        kh = ek / ek.sum(-1, keepdims=True)
        out[:, sl] = qh @ (kh.T @ V[:, sl])
    return (out @ inp["wo"].astype(np.float64)).astype(np.float32)
